# revision 18
# baseline (speedup 1.0000x reference)
"""Trainium2 Bass kernel for nn_EdgeConvolution (gnn_message_passing).

Math
----
Reference (B=2, N=512, C=128, U=128), adj binary {0,1}:
  a_sel[b,i]      = adj[b,i, xidx[b,i]]
  out_pre[b,i,j]  = adj[b,i,j] * (u_i + (a_sel_i - 1)*v_i) + b,  u = x@W1, v = x@W2
Over j there are only two values per (b,i) row:
  zzb_i = u_i + b + (a_sel_i - 1)*v_i = (x@(W1-W2) + b) + a_sel_i * v_i
  z1_i = relu(zzb_i)   (edges with adj=1, count k_i)
  z0   = relu(b)       (edges with adj=0, count N-k_i)
  maxp_i    = max(1[k_i>0]*z1_i, 1[k_i<N]*z0)
  n_i       = k_i*1[any z1_i>0] + (N-k_i)*1[any z0>0]
  avgpool_i = [ k_i*x_i | k_i*(a_sel_i-1)*x_i ] / n_i
Per-core slab: 128 of the 1024 (b,i) rows; w/b replicated.

v2 layout/schedule (vs v1 baseline, 19.6us):
 - adj shipped as fp16 (lossless for {0,1}): halves the dominant DMA; xidx
   rides as fp16 column 0 of the same transfer.
 - x.T / (W1-W2) / W2 / x / b shipped fp16; matmul in fp16 (full PE rate),
   bias folded into PSUM via a ones-row matmul so zzb is ONE stt op
   (v·a_sel + (u+b-v)) instead of three.
 - Both HWDGE rings used: sync carries adj in + out[:,0:192];
   scalar carries brow2/xtw/x in + out[:,192:384].
 - DVE owns the serial chain (k, a_sel, zzb, z1, s1k, nn, rn, final max);
   pool/ACT feed per-row scalars off the critical path.
"""

import numpy as np

B, N, C, U = 2, 512, 128, 128
P = 128          # rows (b,i) per core == SBUF partitions
NCORES = 8
OUTF = U + 2 * C  # 384

_CACHE: dict = {}


def _build_nc():
    import concourse.bacc as bacc
    import concourse.bass as bass
    import concourse.mybir as mybir

    f32 = mybir.dt.float32
    f16 = mybir.dt.float16
    Alu = mybir.AluOpType
    AX = mybir.AxisListType.X
    Act = mybir.ActivationFunctionType

    nc = bacc.Bacc("TRN2", target_bir_lowering=False, debug=False,
                   num_devices=NCORES)

    adj_d = nc.dram_tensor("adjx", [P, N + 1], f16, kind="ExternalInput")
    brow2_d = nc.dram_tensor("brow2", [1, 2 * U], f16, kind="ExternalInput")
    xtw_d = nc.dram_tensor("xtw", [P, 3 * C], f16, kind="ExternalInput")
    x_d = nc.dram_tensor("x", [P, C], f16, kind="ExternalInput")
    out_d = nc.dram_tensor("out", [P, OUTF], f32, kind="ExternalOutput")

    ctx_tensors = [
        ("adj_t", [P, N + 1], f16), ("brow2_t", [1, 2 * U], f16),
        ("xtw_t", [P, 3 * C], f16), ("x_t", [P, C], f16),
        ("ones1", [1, P], f16), ("iota_t", [P, N], f16),
        ("scr", [P, N], f16), ("wscr", [P, 1], f32), ("zcol", [P, 1], f32),
        ("a_sel", [P, 1], f32), ("k", [P, 1], f32), ("asm1", [P, 1], f32),
        ("tsb", [P, U], f32),
        ("zzb", [P, U], f32), ("z1", [P, U], f32), ("z1sum", [P, 1], f32),
        ("s1k", [P, 1], f32), ("nn", [P, 1], f32), ("rn", [P, 1], f32),
        ("z0", [P, U], f32), ("z0sum", [P, 1], f32), ("s0", [P, 1], f32),
        ("nk", [P, 1], f32), ("t2", [P, 1], f32), ("h0", [P, 1], f32),
        ("h1", [P, 1], f32), ("z0h", [P, U], f32),
        ("xk", [P, C], f32), ("xka", [P, C], f32),
        ("out_t", [P, OUTF], f32),
    ]

    from contextlib import ExitStack
    with ExitStack() as ctx:
        t = {}
        for name, shape, dt in ctx_tensors:
            t[name] = ctx.enter_context(nc.sbuf_tensor(name, shape, dt))
        mm = ctx.enter_context(nc.psum_tensor("mm", [P, 2 * U], f32))
        bc = ctx.enter_context(nc.psum_tensor("bc", [P, U], f32))

        dadj = ctx.enter_context(nc.semaphore("dadj"))
        dbr = ctx.enter_context(nc.semaphore("dbr"))
        dxtw = ctx.enter_context(nc.semaphore("dxtw"))
        dx = ctx.enter_context(nc.semaphore("dx"))
        sini = ctx.enter_context(nc.semaphore("sini"))
        siota = ctx.enter_context(nc.semaphore("siota"))
        spe = ctx.enter_context(nc.semaphore("spe"))
        sdve = ctx.enter_context(nc.semaphore("sdve"))
        spool = ctx.enter_context(nc.semaphore("spool"))
        sz0 = ctx.enter_context(nc.semaphore("sz0"))
        sact = ctx.enter_context(nc.semaphore("sact"))
        sfv = ctx.enter_context(nc.semaphore("sfv"))
        sfp = ctx.enter_context(nc.semaphore("sfp"))
        do1 = ctx.enter_context(nc.semaphore("do1"))
        do2 = ctx.enter_context(nc.semaphore("do2"))

        block = ctx.enter_context(nc.Block())

        ap = lambda h: h.ap()

        @block.sync
        def _(sync):
            sync.dma_start(ap(t["adj_t"]), adj_d.ap()).then_inc(dadj, 16)
            sync.wait_ge(sfv, 1)             # out[:,0:128] (DVE final)
            sync.wait_ge(sact, 3)            # out[:,128:256] (ACT final)
            sync.dma_start(out_d.ap()[:, 0:192],
                           t["out_t"].ap()[:, 0:192]).then_inc(do1, 16)
            sync.wait_ge(do1, 16)

        @block.scalar
        def _(act):
            act.dma_start(ap(t["brow2_t"]), brow2_d.ap()).then_inc(dbr, 16)
            act.dma_start(ap(t["xtw_t"]), xtw_d.ap()).then_inc(dxtw, 16)
            act.dma_start(ap(t["x_t"]), x_d.ap()).then_inc(dx, 16)
            # warm the Relu table off the critical path
            act.wait_ge(sini, 2)
            nc.scalar.activation(out=ap(t["wscr"]), in_=ap(t["zcol"]),
                                 func=Act.Relu)
            act.wait_ge(spe, 1)              # bc = ones x b broadcast done
            nc.scalar.activation(out=ap(t["z0"]), in_=bc.ap(), func=Act.Relu,
                                 accum_out=t["z0sum"].ap()[:, 0:1]
                                 ).then_inc(sz0, 1)
            act.wait_ge(dx, 16)
            act.wait_ge(sdve, 1)             # k
            nc.scalar.activation(out=ap(t["xk"]), in_=ap(t["x_t"]),
                                 func=Act.Copy, scale=t["k"].ap()[:, 0:1]
                                 ).then_inc(sact, 1)
            act.wait_ge(spool, 5)            # asm1 (gated: 5 => 1..5 done)
            act.wait_ge(sact, 1)             # xk visible (self)
            nc.scalar.activation(out=ap(t["xka"]), in_=ap(t["xk"]),
                                 func=Act.Copy, scale=t["asm1"].ap()[:, 0:1]
                                 ).then_inc(sact, 1)  # ->2
            act.wait_ge(sdve, 8)             # rn
            nc.scalar.activation(out=t["out_t"].ap()[:, U:U + C],
                                 in_=ap(t["xk"]), func=Act.Copy,
                                 scale=t["rn"].ap()[:, 0:1]).then_inc(sact, 1)  # ->3
            act.wait_ge(sact, 3)             # own final visible (self)
            act.wait_ge(sfp, 1)              # out[:,256:384] (pool final)
            act.dma_start(out_d.ap()[:, 192:384],
                          t["out_t"].ap()[:, 192:384]).then_inc(do2, 16)
            act.wait_ge(do2, 16)

        @block.tensor
        def _(pe):
            pe.wait_ge(sini, 2)              # ones1 ready
            pe.wait_ge(dbr, 16)              # brow2 landed
            nc.tensor.matmul(bc.ap(), lhsT=t["ones1"].ap(),
                             rhs=t["brow2_t"].ap()[0:1, 0:U], start=True,
                             stop=True).then_inc(spe, 1)
            nc.tensor.matmul(mm.ap(), lhsT=t["ones1"].ap(),
                             rhs=ap(t["brow2_t"]), start=True, stop=False)
            pe.wait_ge(dxtw, 16)
            nc.tensor.matmul(mm.ap(), lhsT=t["xtw_t"].ap()[:, 0:C],
                             rhs=t["xtw_t"].ap()[:, C:3 * C], start=False,
                             stop=True).then_inc(spe, 1)  # ->2

        @block.vector
        def _(dve):
            dve.wait_ge(dadj, 16)
            nc.vector.reduce_sum(ap(t["k"]), t["adj_t"].ap()[:, 0:N],
                                 axis=AX).then_inc(sdve, 1)            # ->1
            dve.wait_ge(siota, 1)
            nc.vector.scalar_tensor_tensor(
                out=ap(t["scr"]), in0=ap(t["iota_t"]),
                scalar=t["adj_t"].ap()[:, N:N + 1],
                in1=t["adj_t"].ap()[:, 0:N],
                op0=Alu.is_equal, op1=Alu.mult,
                accum_out=t["a_sel"].ap()[:, 0:1]).then_inc(sdve, 1)   # ->2
            dve.wait_ge(sdve, 2)             # a_sel accum lands async
            dve.wait_ge(spe, 2)              # mm = [u+b-v | v]
            nc.vector.tensor_scalar(out=ap(t["tsb"]),
                                    in0=mm.ap()[:, U:2 * U],
                                    scalar1=t["a_sel"].ap()[:, 0:1],
                                    scalar2=None,
                                    op0=Alu.mult).then_inc(sdve, 1)    # ->3
            dve.wait_ge(sdve, 3)             # tsb visible
            nc.vector.tensor_tensor(ap(t["zzb"]), ap(t["tsb"]),
                                    mm.ap()[:, 0:U],
                                    op=Alu.add).then_inc(sdve, 1)      # ->4
            dve.wait_ge(sdve, 4)             # zzb visible
            nc.vector.tensor_scalar(out=ap(t["z1"]), in0=ap(t["zzb"]),
                                    scalar1=0.0, scalar2=None, op0=Alu.max,
                                    op1=Alu.add,
                                    accum_out=t["z1sum"].ap()[:, 0:1]
                                    ).then_inc(sdve, 1)                # ->5
            dve.wait_ge(sdve, 5)             # z1sum accum landed
            nc.vector.tensor_scalar(out=ap(t["s1k"]), in0=ap(t["z1sum"]),
                                    scalar1=0.0,
                                    scalar2=t["k"].ap()[:, 0:1],
                                    op0=Alu.is_gt,
                                    op1=Alu.mult).then_inc(sdve, 1)    # ->6
            dve.wait_ge(sdve, 6)             # s1k visible
            dve.wait_ge(spool, 7)            # t2 (+ z0h, h1 for the final)
            nc.vector.tensor_tensor(ap(t["nn"]), ap(t["s1k"]), ap(t["t2"]),
                                    op=Alu.add).then_inc(sdve, 1)      # ->7
            dve.wait_ge(sdve, 7)             # nn visible
            nc.vector.reciprocal(ap(t["rn"]), ap(t["nn"])).then_inc(sdve, 1)  # ->8
            dve.wait_ge(sdve, 8)             # rn visible
            nc.vector.scalar_tensor_tensor(
                out=t["out_t"].ap()[:, 0:U], in0=ap(t["z1"]),
                scalar=t["h1"].ap()[:, 0:1], in1=ap(t["z0h"]),
                op0=Alu.mult, op1=Alu.max).then_inc(sfv, 1)

        @block.gpsimd
        def _(pool):
            nc.gpsimd.memset(ap(t["zcol"]), 0.0).then_inc(sini, 1)
            nc.gpsimd.memset(ap(t["ones1"]), 1.0).then_inc(sini, 1)  # ->2
            nc.gpsimd.iota(ap(t["iota_t"]), pattern=[[1, N]], base=0,
                           channel_multiplier=0,
                           allow_small_or_imprecise_dtypes=True
                           ).then_inc(siota, 1)
            pool.wait_ge(sdve, 1)            # k
            nc.gpsimd.tensor_scalar(out=ap(t["h1"]), in0=ap(t["k"]),
                                    scalar1=0.0, scalar2=None,
                                    op0=Alu.is_gt).then_inc(spool, 1)  # ->1
            nc.gpsimd.tensor_scalar(out=ap(t["h0"]), in0=ap(t["k"]),
                                    scalar1=float(N), scalar2=None,
                                    op0=Alu.is_lt).then_inc(spool, 1)  # ->2
            nc.gpsimd.tensor_scalar(out=ap(t["nk"]), in0=ap(t["k"]),
                                    scalar1=-1.0, scalar2=float(N),
                                    op0=Alu.mult,
                                    op1=Alu.add).then_inc(spool, 1)    # ->3
            pool.wait_ge(sdve, 2)            # a_sel
            nc.gpsimd.tensor_scalar(out=ap(t["asm1"]), in0=ap(t["a_sel"]),
                                    scalar1=-1.0, scalar2=None,
                                    op0=Alu.add).then_inc(spool, 1)    # ->4
            pool.wait_ge(sz0, 1)             # z0/z0sum
            nc.gpsimd.tensor_scalar(out=ap(t["s0"]), in0=ap(t["z0sum"]),
                                    scalar1=0.0, scalar2=None,
                                    op0=Alu.is_gt).then_inc(spool, 1)  # ->5
            pool.wait_ge(spool, 5)           # nk, s0, h0 all landed
            nc.gpsimd.tensor_mul(ap(t["t2"]), ap(t["nk"]),
                                 ap(t["s0"])).then_inc(spool, 1)       # ->6
            nc.gpsimd.tensor_scalar(out=ap(t["z0h"]), in0=ap(t["z0"]),
                                    scalar1=t["h0"].ap()[:, 0:1],
                                    scalar2=None,
                                    op0=Alu.mult).then_inc(spool, 1)   # ->7
            pool.wait_ge(sact, 2)            # xka
            pool.wait_ge(sdve, 8)            # rn
            nc.gpsimd.tensor_scalar(out=t["out_t"].ap()[:, U + C:OUTF],
                                    in0=ap(t["xka"]),
                                    scalar1=t["rn"].ap()[:, 0:1],
                                    scalar2=None,
                                    op0=Alu.mult).then_inc(sfp, 1)

    nc.compile()
    return nc


def get_nc():
    if "nc" not in _CACHE:
        _CACHE["nc"] = _build_nc()
    return _CACHE["nc"]


def make_in_maps(inputs, adj_matrix, xidx, w, b):
    """Shard full inputs into per-core input maps (128 (b,i) rows per core)."""
    x_flat = np.asarray(inputs, dtype=np.float32).reshape(B * N, C)
    adj_flat = np.asarray(adj_matrix, dtype=np.float32).reshape(B * N, N)
    xidx_flat = np.asarray(xidx, dtype=np.int32).reshape(B * N)
    w_full = np.asarray(w, dtype=np.float32)[0]          # [2C, U]
    w1 = w_full[0:C]
    w2 = w_full[C:2 * C]
    wd = (w1 - w2).astype(np.float16)
    w2h = w2.astype(np.float16)
    brow2 = np.zeros((1, 2 * U), dtype=np.float16)
    brow2[0, 0:U] = np.asarray(b, dtype=np.float32)

    in_maps = []
    for c in range(NCORES):
        rows = slice(c * P, (c + 1) * P)
        x_slab = x_flat[rows]
        adjx = np.empty((P, N + 1), dtype=np.float16)
        adjx[:, 0:N] = adj_flat[rows]
        adjx[:, N] = xidx_flat[rows]
        xtw = np.empty((P, 3 * C), dtype=np.float16)
        xtw[:, 0:C] = x_slab.T
        xtw[:, C:2 * C] = wd
        xtw[:, 2 * C:3 * C] = w2h
        in_maps.append({
            "adjx": adjx,
            "brow2": brow2,
            "xtw": xtw,
            "x": x_slab.astype(np.float16),
        })
    return in_maps


def kernel(inputs, adj_matrix, xidx, w, b, _trace=False):
    from concourse.bass_utils import run_bass_kernel_spmd

    nc = get_nc()
    in_maps = make_in_maps(inputs, adj_matrix, xidx, w, b)
    res = run_bass_kernel_spmd(nc, in_maps, list(range(NCORES)),
                               trace=_trace)
    out = np.concatenate([res.results[c]["out"] for c in range(NCORES)],
                         axis=0)
    out = out.reshape(B, N, OUTF).astype(np.float32)
    if _trace:
        _CACHE["last_results"] = res
    return out
